# revision 12
# baseline (speedup 1.0000x reference)
"""MoE (top-2 of 8 experts, SwiGLU FFN) on 8 Trainium2 NeuronCores.

Strategy: expert-parallel. The gate/top-k routing is computed on host
(bit-exact with the reference: jax on CPU), tokens are dispatched to the
core owning their expert (sharding by top-k index), each core runs a
dense SwiGLU FFN over its gathered tokens (bf16 matmuls, fp32
accumulation) and scales rows by the renormalized top-k weight. The host
scatter-adds the per-expert partial outputs into the full [B,S,D] output.

Problem dims (hardcoded): B=4, S=2048, D=1024, E=8, TOP_K=2, H=3072.

SBUF budget per partition (bytes): w1+w3 96K, w2 48K, g 24K, xg 8K,
sil 4K, ot 4K  -> ~184K of 192K.
PSUM: ps1 x2 + ps3 x2 (stage A) + pso x4 (stage B) = 8 banks.
"""

import sys

if "/opt/trn_rl_repo" not in sys.path:
    sys.path.insert(0, "/opt/trn_rl_repo")

import numpy as np
import ml_dtypes

B, S, D = 4, 2048, 1024
E = 8
TOP_K = 2
H = 3 * D
T = B * S

BF16 = ml_dtypes.bfloat16

_nc_cache: dict = {}


def build_expert_ffn(C: int):
    """Bass program for one core: dense SwiGLU FFN over C gathered tokens.

    Inputs (per core e):
      xgT [D, C]  bf16 : gathered tokens, transposed, zero-padded
      w1  [D, H]  bf16
      w3  [D, H]  bf16
      w2  [H, D]  bf16
      wts [C, 1]  f32  : per-token combine weight (0 for padding)
    Output:
      yg  [C, D]  f32  : wts * (silu(xg@w1) * (xg@w3)) @ w2
    """
    import concourse.bacc as bacc
    import concourse.tile as tile
    import concourse.mybir as mybir

    fp32 = mybir.dt.float32
    bf16 = mybir.dt.bfloat16

    assert C % 128 == 0
    # token chunks (PSUM bank free dim <= 512); keep >=256 where possible so
    # the [128,128] LDWEIGHTS stays hidden under the matmul stream
    chunks = []
    rem = C
    while rem > 640:
        chunks.append(512)
        rem -= 512
    if rem <= 512:
        chunks.append(rem)
    else:
        chunks.extend([rem - 256, 256])
    KD = D // 128                # 8  k-tiles over D
    NH = H // 128                # 24 h-tiles over H
    ND = D // 512                # 2  512-wide output column tiles

    nc = bacc.Bacc("TRN2", target_bir_lowering=False, debug=False, num_devices=8)

    xgT = nc.dram_tensor("xgT", [D, C], bf16, kind="ExternalInput")
    w1 = nc.dram_tensor("w1", [D, H], bf16, kind="ExternalInput")
    w3 = nc.dram_tensor("w3", [D, H], bf16, kind="ExternalInput")
    w2 = nc.dram_tensor("w2", [H, D], bf16, kind="ExternalInput")
    # combine weights pre-tiled on host: wts[p, n] = weight of token n*128+p
    wts = nc.dram_tensor("wts", [128, C // 128], fp32, kind="ExternalInput")
    yg = nc.dram_tensor("yg", [C, D], fp32, kind="ExternalOutput")

    with tile.TileContext(nc) as tc:
        with (
            tc.tile_pool(name="wres", bufs=1) as wres,
            tc.tile_pool(name="xgp", bufs=1) as xgp,
            tc.tile_pool(name="gp", bufs=1) as gp,
            tc.tile_pool(name="tmp", bufs=2) as tmp,
            tc.tile_pool(name="outp", bufs=2) as outp,
            tc.tile_pool(name="psA", bufs=2, space="PSUM") as psA,
            tc.tile_pool(name="psB", bufs=4, space="PSUM") as psB,
        ):
            # DMA issue order matters: the first chunk's tokens and the
            # combine weights go first (gpsimd/SWDGE queue), then w1/w3
            # (needed by stage A from t~0), then w2 (needed ~80us in).
            def load_xg_chunk(c0, NC):
                xg_sb = []
                for k in range(KD):
                    xt = xgp.tile([128, NC], bf16, tag=f"xg_{k}")
                    nc.gpsimd.dma_start(
                        xt[:], xgT.ap()[k * 128:(k + 1) * 128, c0:c0 + NC]
                    )
                    xg_sb.append(xt)
                return xg_sb

            xg0_sb = load_xg_chunk(0, chunks[0])
            wts_sb = wres.tile([128, C // 128], fp32, tag="wts")
            nc.gpsimd.dma_start(wts_sb[:], wts.ap())

            # resident weights, lhsT layouts; spread across DMA queues so
            # chunk 0's stage A isn't serialized behind one queue
            w1_sb = []
            w3_sb = []
            for k in range(KD):
                t1 = wres.tile([128, H], bf16, tag=f"w1_{k}")
                nc.sync.dma_start(t1[:], w1.ap()[k * 128:(k + 1) * 128, :])
                w1_sb.append(t1)
                t3 = wres.tile([128, H], bf16, tag=f"w3_{k}")
                nc.scalar.dma_start(t3[:], w3.ap()[k * 128:(k + 1) * 128, :])
                w3_sb.append(t3)
            w2_sb = []
            for ht in range(NH):
                t2 = wres.tile([128, D], bf16, tag=f"w2_{ht}")
                nc.sync.dma_start(t2[:], w2.ap()[ht * 128:(ht + 1) * 128, :])
                w2_sb.append(t2)

            c0 = 0
            for ch, NC in enumerate(chunks):
                NT = NC // 128
                xg_sb = xg0_sb if ch == 0 else load_xg_chunk(c0, NC)

                # stage A: gT[h, tok] = silu(y1) * y3 for all 24 h-tiles
                g_tiles = []
                for ht in range(NH):
                    ps1 = psA.tile([128, NC], fp32, tag="ps1")
                    ps3 = psA.tile([128, NC], fp32, tag="ps3")
                    for k in range(KD):
                        nc.tensor.matmul(
                            ps1[:],
                            w1_sb[k][:, ht * 128:(ht + 1) * 128],
                            xg_sb[k][:],
                            start=(k == 0),
                            stop=(k == KD - 1),
                        )
                    for k in range(KD):
                        nc.tensor.matmul(
                            ps3[:],
                            w3_sb[k][:, ht * 128:(ht + 1) * 128],
                            xg_sb[k][:],
                            start=(k == 0),
                            stop=(k == KD - 1),
                        )
                    sig = tmp.tile([128, NC], fp32, tag="sig")
                    nc.scalar.activation(
                        sig[:], ps1[:], mybir.ActivationFunctionType.Sigmoid
                    )
                    sil = tmp.tile([128, NC], fp32, tag="sil")
                    nc.vector.tensor_mul(sil[:], sig[:], ps1[:])
                    gt = gp.tile([128, NC], bf16, tag=f"g_{ht}")
                    nc.vector.tensor_mul(gt[:], sil[:], ps3[:])
                    g_tiles.append(gt)

                # stage B: yg[tok, d] = wts[tok] * (g.T @ w2)
                for tt in range(NT):
                    gtile_idx = c0 // 128 + tt
                    for dh in range(ND):
                        pso = psB.tile([128, 512], fp32, tag="pso")
                        for ht in range(NH):
                            nc.tensor.matmul(
                                pso[:],
                                g_tiles[ht][:, tt * 128:(tt + 1) * 128],
                                w2_sb[ht][:, dh * 512:(dh + 1) * 512],
                                start=(ht == 0),
                                stop=(ht == NH - 1),
                            )
                        ot = outp.tile([128, 512], fp32, tag="ot")
                        nc.vector.tensor_scalar_mul(
                            ot[:], pso[:], wts_sb[:, gtile_idx:gtile_idx + 1]
                        )
                        nc.sync.dma_start(
                            yg.ap()[
                                c0 + tt * 128: c0 + (tt + 1) * 128,
                                dh * 512:(dh + 1) * 512,
                            ],
                            ot[:],
                        )
                c0 += NC

    nc.compile()
    return nc


def route_host(xf: np.ndarray, gate_w: np.ndarray):
    """Top-2 routing, bit-exact with the reference (jax on CPU)."""
    import jax
    import jax.numpy as jnp

    cpu = jax.devices("cpu")[0]
    with jax.default_device(cpu):
        xj = jax.device_put(xf, cpu)
        gj = jax.device_put(gate_w, cpu)
        probs = jax.nn.softmax(xj @ gj, axis=-1)
        vals, idx = jax.lax.top_k(probs, TOP_K)
        w = vals / jnp.sum(vals, axis=-1, keepdims=True)
    return np.asarray(idx), np.asarray(w)


def prepare_dispatch(x, gate_w):
    """Host routing + per-expert gather lists."""
    xf = np.ascontiguousarray(np.asarray(x).reshape(T, D), dtype=np.float32)
    gate_w = np.asarray(gate_w, dtype=np.float32)
    idx, w = route_host(xf, gate_w)
    tok_flat = np.repeat(np.arange(T), TOP_K)
    idx_flat = idx.ravel()
    w_flat = w.astype(np.float32).ravel()
    toks = []
    wts_list = []
    for e in range(E):
        sel = idx_flat == e
        toks.append(tok_flat[sel])
        wts_list.append(w_flat[sel])
    max_n = max(len(t) for t in toks)
    C = max(256, ((max_n + 127) // 128) * 128)
    return xf, toks, wts_list, C


def make_in_maps(xf, toks, wts_list, C, w1, w2, w3):
    xf_bf = xf.astype(BF16)
    in_maps = []
    for e in range(E):
        n_e = len(toks[e])
        xgT = np.zeros((D, C), dtype=BF16)
        xgT[:, :n_e] = xf_bf[toks[e]].T
        wflat = np.zeros(C, dtype=np.float32)
        wflat[:n_e] = wts_list[e]
        wts = np.ascontiguousarray(wflat.reshape(C // 128, 128).T)
        in_maps.append(
            {
                "xgT": xgT,
                "w1": np.asarray(w1[e], dtype=np.float32).astype(BF16),
                "w3": np.asarray(w3[e], dtype=np.float32).astype(BF16),
                "w2": np.asarray(w2[e], dtype=np.float32).astype(BF16),
                "wts": wts,
            }
        )
    return in_maps


def combine_outputs(results, toks):
    out = np.zeros((T, D), dtype=np.float32)
    for e in range(E):
        n_e = len(toks[e])
        out[toks[e]] += np.asarray(results[e]["yg"][:n_e], dtype=np.float32)
    return out.reshape(B, S, D)


def run(x, gate_w, w1, w2, w3, **spmd_kwargs):
    """Run the MoE. Returns (output, BassKernelResults)."""
    from concourse import bass_utils

    xf, toks, wts_list, C = prepare_dispatch(x, gate_w)
    if C not in _nc_cache:
        _nc_cache[C] = build_expert_ffn(C)
    nc = _nc_cache[C]

    in_maps = make_in_maps(xf, toks, wts_list, C, w1, w2, w3)
    res = bass_utils.run_bass_kernel_spmd(
        nc, in_maps, core_ids=list(range(E)), **spmd_kwargs
    )
    out = combine_outputs(res.results, toks).astype(np.asarray(x).dtype, copy=False)
    return out, res


def kernel(x, gate_w, w1, w2, w3):
    out, _ = run(x, gate_w, w1, w2, w3)
    return out


# revision 14
# speedup vs baseline: 1.1579x; 1.1579x over previous
"""MoE (top-2 of 8 experts, SwiGLU FFN) on 8 Trainium2 NeuronCores.

Strategy: expert-parallel. The gate/top-k routing is computed on host
(bit-exact with the reference: jax on CPU), tokens are dispatched to the
core owning their expert (sharding by top-k index), each core runs a
dense SwiGLU FFN over its gathered tokens (bf16 matmuls, fp32
accumulation) and scales rows by the renormalized top-k weight. The host
scatter-adds the per-expert partial outputs into the full [B,S,D] output.

Problem dims (hardcoded): B=4, S=2048, D=1024, E=8, TOP_K=2, H=3072.

SBUF budget per partition (bytes): w1+w3 96K, w2 48K, g 24K, xg 8K,
sil 4K, ot 4K  -> ~184K of 192K.
PSUM: ps1 x2 + ps3 x2 (stage A) + pso x4 (stage B) = 8 banks.
"""

import sys

if "/opt/trn_rl_repo" not in sys.path:
    sys.path.insert(0, "/opt/trn_rl_repo")

import numpy as np
import ml_dtypes

B, S, D = 4, 2048, 1024
E = 8
TOP_K = 2
H = 3 * D
T = B * S

BF16 = ml_dtypes.bfloat16

_nc_cache: dict = {}


def build_expert_ffn(C: int):
    """Bass program for one core: dense SwiGLU FFN over C gathered tokens.

    Inputs (per core e):
      xgT [D, C]  bf16 : gathered tokens, transposed, zero-padded
      w1  [D, H]  bf16
      w3  [D, H]  bf16
      w2  [H, D]  bf16
      wts [C, 1]  f32  : per-token combine weight (0 for padding)
    Output:
      yg  [C, D]  f32  : wts * (silu(xg@w1) * (xg@w3)) @ w2
    """
    import concourse.bacc as bacc
    import concourse.tile as tile
    import concourse.mybir as mybir

    fp32 = mybir.dt.float32
    bf16 = mybir.dt.bfloat16

    assert C % 128 == 0
    # token chunks (PSUM bank free dim <= 512); keep >=256 where possible so
    # the [128,128] LDWEIGHTS stays hidden under the matmul stream
    chunks = []
    rem = C
    while rem > 640:
        chunks.append(512)
        rem -= 512
    if rem <= 512:
        chunks.append(rem)
    else:
        chunks.extend([rem - 256, 256])
    KD = D // 128                # 8  k-tiles over D
    NH = H // 128                # 24 h-tiles over H
    ND = D // 512                # 2  512-wide output column tiles

    nc = bacc.Bacc("TRN2", target_bir_lowering=False, debug=False, num_devices=8)

    xgT = nc.dram_tensor("xgT", [D, C], bf16, kind="ExternalInput")
    w1 = nc.dram_tensor("w1", [D, H], bf16, kind="ExternalInput")
    w3 = nc.dram_tensor("w3", [D, H], bf16, kind="ExternalInput")
    w2 = nc.dram_tensor("w2", [H, D], bf16, kind="ExternalInput")
    # combine weights pre-tiled on host: wts[p, n] = weight of token n*128+p
    wts = nc.dram_tensor("wts", [128, C // 128], fp32, kind="ExternalInput")
    yg = nc.dram_tensor("yg", [C, D], fp32, kind="ExternalOutput")

    with tile.TileContext(nc) as tc:
        with (
            tc.tile_pool(name="wres", bufs=1) as wres,
            tc.tile_pool(name="xgp", bufs=1) as xgp,
            tc.tile_pool(name="gp", bufs=1) as gp,
            tc.tile_pool(name="tmp", bufs=2) as tmp,
            tc.tile_pool(name="outp", bufs=2) as outp,
            tc.tile_pool(name="psA", bufs=2, space="PSUM") as psA,
            tc.tile_pool(name="psB", bufs=4, space="PSUM") as psB,
        ):
            # DMA issue order matters: the first chunk's tokens and the
            # combine weights go first (gpsimd/SWDGE queue), then w1/w3
            # (needed by stage A from t~0), then w2 (needed ~80us in).
            def load_xg_chunk(c0, NC, eng=None):
                eng = eng or nc.gpsimd
                xg_sb = []
                for k in range(KD):
                    xt = xgp.tile([128, NC], bf16, tag=f"xg_{k}")
                    eng.dma_start(
                        xt[:], xgT.ap()[k * 128:(k + 1) * 128, c0:c0 + NC]
                    )
                    xg_sb.append(xt)
                return xg_sb

            # sync (HWDGE) queue in first-use order: chunk-0 tokens, combine
            # weights, then w1/w3 interleaved. w2 goes on the gpsimd queue in
            # parallel (not needed until stage B, ~80us in).
            xg0_sb = load_xg_chunk(0, chunks[0], eng=nc.sync)
            wts_sb = wres.tile([128, C // 128], fp32, tag="wts")
            nc.sync.dma_start(wts_sb[:], wts.ap())

            w1_sb = []
            w3_sb = []
            for k in range(KD):
                t1 = wres.tile([128, H], bf16, tag=f"w1_{k}")
                nc.sync.dma_start(t1[:], w1.ap()[k * 128:(k + 1) * 128, :])
                w1_sb.append(t1)
                t3 = wres.tile([128, H], bf16, tag=f"w3_{k}")
                nc.sync.dma_start(t3[:], w3.ap()[k * 128:(k + 1) * 128, :])
                w3_sb.append(t3)
            w2_sb = []
            for ht in range(NH):
                t2 = wres.tile([128, D], bf16, tag=f"w2_{ht}")
                nc.gpsimd.dma_start(t2[:], w2.ap()[ht * 128:(ht + 1) * 128, :])
                w2_sb.append(t2)

            c0 = 0
            for ch, NC in enumerate(chunks):
                NT = NC // 128
                xg_sb = xg0_sb if ch == 0 else load_xg_chunk(c0, NC)

                # stage A: gT[h, tok] = silu(y1) * y3 for all 24 h-tiles
                g_tiles = []
                for ht in range(NH):
                    # chunk 0: stage B is idle, so borrow psB's banks for
                    # extra in-flight groups while w1/w3 are still arriving
                    pool = psB if (ch == 0 and ht % 2 == 1) else psA
                    ps1 = pool.tile([128, NC], fp32, tag="ps1" if pool is psA else "pso")
                    ps3 = pool.tile([128, NC], fp32, tag="ps3" if pool is psA else "pso")
                    for k in range(KD):
                        nc.tensor.matmul(
                            ps1[:],
                            w1_sb[k][:, ht * 128:(ht + 1) * 128],
                            xg_sb[k][:],
                            start=(k == 0),
                            stop=(k == KD - 1),
                        )
                    for k in range(KD):
                        nc.tensor.matmul(
                            ps3[:],
                            w3_sb[k][:, ht * 128:(ht + 1) * 128],
                            xg_sb[k][:],
                            start=(k == 0),
                            stop=(k == KD - 1),
                        )
                    sig = tmp.tile([128, NC], fp32, tag="sig")
                    nc.scalar.activation(
                        sig[:], ps1[:], mybir.ActivationFunctionType.Sigmoid
                    )
                    sil = tmp.tile([128, NC], fp32, tag="sil")
                    nc.vector.tensor_mul(sil[:], sig[:], ps1[:])
                    gt = gp.tile([128, NC], bf16, tag=f"g_{ht}")
                    nc.vector.tensor_mul(gt[:], sil[:], ps3[:])
                    g_tiles.append(gt)

                # stage B: yg[tok, d] = wts[tok] * (g.T @ w2)
                for tt in range(NT):
                    gtile_idx = c0 // 128 + tt
                    for dh in range(ND):
                        pso = psB.tile([128, 512], fp32, tag="pso")
                        for ht in range(NH):
                            nc.tensor.matmul(
                                pso[:],
                                g_tiles[ht][:, tt * 128:(tt + 1) * 128],
                                w2_sb[ht][:, dh * 512:(dh + 1) * 512],
                                start=(ht == 0),
                                stop=(ht == NH - 1),
                            )
                        ot = outp.tile([128, 512], fp32, tag="ot")
                        nc.vector.tensor_scalar_mul(
                            ot[:], pso[:], wts_sb[:, gtile_idx:gtile_idx + 1]
                        )
                        nc.sync.dma_start(
                            yg.ap()[
                                c0 + tt * 128: c0 + (tt + 1) * 128,
                                dh * 512:(dh + 1) * 512,
                            ],
                            ot[:],
                        )
                c0 += NC

    nc.compile()
    return nc


def route_host(xf: np.ndarray, gate_w: np.ndarray):
    """Top-2 routing, bit-exact with the reference (jax on CPU)."""
    import jax
    import jax.numpy as jnp

    cpu = jax.devices("cpu")[0]
    with jax.default_device(cpu):
        xj = jax.device_put(xf, cpu)
        gj = jax.device_put(gate_w, cpu)
        probs = jax.nn.softmax(xj @ gj, axis=-1)
        vals, idx = jax.lax.top_k(probs, TOP_K)
        w = vals / jnp.sum(vals, axis=-1, keepdims=True)
    return np.asarray(idx), np.asarray(w)


def prepare_dispatch(x, gate_w):
    """Host routing + per-expert gather lists."""
    xf = np.ascontiguousarray(np.asarray(x).reshape(T, D), dtype=np.float32)
    gate_w = np.asarray(gate_w, dtype=np.float32)
    idx, w = route_host(xf, gate_w)
    tok_flat = np.repeat(np.arange(T), TOP_K)
    idx_flat = idx.ravel()
    w_flat = w.astype(np.float32).ravel()
    toks = []
    wts_list = []
    for e in range(E):
        sel = idx_flat == e
        toks.append(tok_flat[sel])
        wts_list.append(w_flat[sel])
    max_n = max(len(t) for t in toks)
    C = max(256, ((max_n + 127) // 128) * 128)
    return xf, toks, wts_list, C


def make_in_maps(xf, toks, wts_list, C, w1, w2, w3):
    xf_bf = xf.astype(BF16)
    in_maps = []
    for e in range(E):
        n_e = len(toks[e])
        xgT = np.zeros((D, C), dtype=BF16)
        xgT[:, :n_e] = xf_bf[toks[e]].T
        wflat = np.zeros(C, dtype=np.float32)
        wflat[:n_e] = wts_list[e]
        wts = np.ascontiguousarray(wflat.reshape(C // 128, 128).T)
        in_maps.append(
            {
                "xgT": xgT,
                "w1": np.asarray(w1[e], dtype=np.float32).astype(BF16),
                "w3": np.asarray(w3[e], dtype=np.float32).astype(BF16),
                "w2": np.asarray(w2[e], dtype=np.float32).astype(BF16),
                "wts": wts,
            }
        )
    return in_maps


def combine_outputs(results, toks):
    out = np.zeros((T, D), dtype=np.float32)
    for e in range(E):
        n_e = len(toks[e])
        out[toks[e]] += np.asarray(results[e]["yg"][:n_e], dtype=np.float32)
    return out.reshape(B, S, D)


def run(x, gate_w, w1, w2, w3, **spmd_kwargs):
    """Run the MoE. Returns (output, BassKernelResults)."""
    from concourse import bass_utils

    xf, toks, wts_list, C = prepare_dispatch(x, gate_w)
    if C not in _nc_cache:
        _nc_cache[C] = build_expert_ffn(C)
    nc = _nc_cache[C]

    in_maps = make_in_maps(xf, toks, wts_list, C, w1, w2, w3)
    res = bass_utils.run_bass_kernel_spmd(
        nc, in_maps, core_ids=list(range(E)), **spmd_kwargs
    )
    out = combine_outputs(res.results, toks).astype(np.asarray(x).dtype, copy=False)
    return out, res


def kernel(x, gate_w, w1, w2, w3):
    out, _ = run(x, gate_w, w1, w2, w3)
    return out


# revision 15
# speedup vs baseline: 1.2033x; 1.0392x over previous
"""MoE (top-2 of 8 experts, SwiGLU FFN) on 8 Trainium2 NeuronCores.

Strategy: expert-parallel. The gate/top-k routing is computed on host
(bit-exact with the reference: jax on CPU), tokens are dispatched to the
core owning their expert (sharding by top-k index), each core runs a
dense SwiGLU FFN over its gathered tokens (bf16 matmuls, fp32
accumulation) and scales rows by the renormalized top-k weight. The host
scatter-adds the per-expert partial outputs into the full [B,S,D] output.

Problem dims (hardcoded): B=4, S=2048, D=1024, E=8, TOP_K=2, H=3072.

SBUF budget per partition (bytes): w1+w3 96K, w2 48K, g 24K, xg 8K,
sil 4K, ot 4K  -> ~184K of 192K.
PSUM: ps1 x2 + ps3 x2 (stage A) + pso x4 (stage B) = 8 banks.
"""

import sys

if "/opt/trn_rl_repo" not in sys.path:
    sys.path.insert(0, "/opt/trn_rl_repo")

import numpy as np
import ml_dtypes

B, S, D = 4, 2048, 1024
E = 8
TOP_K = 2
H = 3 * D
T = B * S

BF16 = ml_dtypes.bfloat16

_nc_cache: dict = {}


def build_expert_ffn(C: int):
    """Bass program for one core: dense SwiGLU FFN over C gathered tokens.

    Inputs (per core e):
      xgT [D, C]  bf16 : gathered tokens, transposed, zero-padded
      w1  [D, H]  bf16
      w3  [D, H]  bf16
      w2  [H, D]  bf16
      wts [C, 1]  f32  : per-token combine weight (0 for padding)
    Output:
      yg  [C, D]  f32  : wts * (silu(xg@w1) * (xg@w3)) @ w2
    """
    import concourse.bacc as bacc
    import concourse.tile as tile
    import concourse.mybir as mybir

    fp32 = mybir.dt.float32
    bf16 = mybir.dt.bfloat16

    assert C % 128 == 0
    # token chunks (PSUM bank free dim <= 512); keep >=256 where possible so
    # the [128,128] LDWEIGHTS stays hidden under the matmul stream
    chunks = []
    rem = C
    while rem > 640:
        chunks.append(512)
        rem -= 512
    if rem <= 512:
        chunks.append(rem)
    else:
        chunks.extend([rem - 256, 256])
    KD = D // 128                # 8  k-tiles over D
    NH = H // 128                # 24 h-tiles over H
    ND = D // 512                # 2  512-wide output column tiles

    nc = bacc.Bacc("TRN2", target_bir_lowering=False, debug=False, num_devices=8)

    xgT = nc.dram_tensor("xgT", [D, C], bf16, kind="ExternalInput")
    w1 = nc.dram_tensor("w1", [D, H], bf16, kind="ExternalInput")
    w3 = nc.dram_tensor("w3", [D, H], bf16, kind="ExternalInput")
    w2 = nc.dram_tensor("w2", [H, D], bf16, kind="ExternalInput")
    # combine weights pre-tiled on host: wts[p, n] = weight of token n*128+p
    wts = nc.dram_tensor("wts", [128, C // 128], fp32, kind="ExternalInput")
    yg = nc.dram_tensor("yg", [C, D], fp32, kind="ExternalOutput")

    with tile.TileContext(nc) as tc:
        with (
            tc.tile_pool(name="wres", bufs=1) as wres,
            tc.tile_pool(name="xgp", bufs=1) as xgp,
            tc.tile_pool(name="gp", bufs=1) as gp,
            tc.tile_pool(name="tmp", bufs=2) as tmp,
            tc.tile_pool(name="outp", bufs=2) as outp,
            tc.tile_pool(name="psA", bufs=2, space="PSUM") as psA,
            tc.tile_pool(name="psB", bufs=4, space="PSUM") as psB,
        ):
            # DMA issue order matters: the first chunk's tokens and the
            # combine weights go first (gpsimd/SWDGE queue), then w1/w3
            # (needed by stage A from t~0), then w2 (needed ~80us in).
            def load_xg_chunk(c0, NC, eng=None):
                eng = eng or nc.gpsimd
                xg_sb = []
                for k in range(KD):
                    xt = xgp.tile([128, NC], bf16, tag=f"xg_{k}")
                    eng.dma_start(
                        xt[:], xgT.ap()[k * 128:(k + 1) * 128, c0:c0 + NC]
                    )
                    xg_sb.append(xt)
                return xg_sb

            # DMA ordering: all queues stripe over the same 16 HW engines, so
            # what matters is issue order by first use. gpsimd: wts + chunk-0
            # tokens (small, early). sync: w1/w3 (stage A, critical from t=0),
            # then w2 (stage B, needed ~80us in).
            wts_sb = wres.tile([128, C // 128], fp32, tag="wts")
            nc.gpsimd.dma_start(wts_sb[:], wts.ap())
            xg0_sb = load_xg_chunk(0, chunks[0])

            w1_sb = []
            w3_sb = []
            for k in range(KD):
                t1 = wres.tile([128, H], bf16, tag=f"w1_{k}")
                nc.sync.dma_start(t1[:], w1.ap()[k * 128:(k + 1) * 128, :])
                w1_sb.append(t1)
                t3 = wres.tile([128, H], bf16, tag=f"w3_{k}")
                nc.sync.dma_start(t3[:], w3.ap()[k * 128:(k + 1) * 128, :])
                w3_sb.append(t3)
            w2_sb = []
            for ht in range(NH):
                t2 = wres.tile([128, D], bf16, tag=f"w2_{ht}")
                nc.sync.dma_start(t2[:], w2.ap()[ht * 128:(ht + 1) * 128, :])
                w2_sb.append(t2)

            c0 = 0
            for ch, NC in enumerate(chunks):
                NT = NC // 128
                xg_sb = xg0_sb if ch == 0 else load_xg_chunk(c0, NC)

                # stage A: gT[h, tok] = silu(y1) * y3 for all 24 h-tiles
                g_tiles = []
                for ht in range(NH):
                    # chunk 0: stage B is idle, so borrow psB's banks for
                    # extra in-flight groups while w1/w3 are still arriving
                    pool = psB if (ch == 0 and ht % 2 == 1) else psA
                    ps1 = pool.tile([128, NC], fp32, tag="ps1" if pool is psA else "pso")
                    ps3 = pool.tile([128, NC], fp32, tag="ps3" if pool is psA else "pso")
                    for k in range(KD):
                        nc.tensor.matmul(
                            ps1[:],
                            w1_sb[k][:, ht * 128:(ht + 1) * 128],
                            xg_sb[k][:],
                            start=(k == 0),
                            stop=(k == KD - 1),
                        )
                    for k in range(KD):
                        nc.tensor.matmul(
                            ps3[:],
                            w3_sb[k][:, ht * 128:(ht + 1) * 128],
                            xg_sb[k][:],
                            start=(k == 0),
                            stop=(k == KD - 1),
                        )
                    sig = tmp.tile([128, NC], fp32, tag="sig")
                    nc.scalar.activation(
                        sig[:], ps1[:], mybir.ActivationFunctionType.Sigmoid
                    )
                    sil = tmp.tile([128, NC], fp32, tag="sil")
                    nc.vector.tensor_mul(sil[:], sig[:], ps1[:])
                    gt = gp.tile([128, NC], bf16, tag=f"g_{ht}")
                    nc.vector.tensor_mul(gt[:], sil[:], ps3[:])
                    g_tiles.append(gt)

                # stage B: yg[tok, d] = wts[tok] * (g.T @ w2)
                for tt in range(NT):
                    gtile_idx = c0 // 128 + tt
                    for dh in range(ND):
                        pso = psB.tile([128, 512], fp32, tag="pso")
                        for ht in range(NH):
                            nc.tensor.matmul(
                                pso[:],
                                g_tiles[ht][:, tt * 128:(tt + 1) * 128],
                                w2_sb[ht][:, dh * 512:(dh + 1) * 512],
                                start=(ht == 0),
                                stop=(ht == NH - 1),
                            )
                        ot = outp.tile([128, 512], fp32, tag="ot")
                        nc.vector.tensor_scalar_mul(
                            ot[:], pso[:], wts_sb[:, gtile_idx:gtile_idx + 1]
                        )
                        nc.sync.dma_start(
                            yg.ap()[
                                c0 + tt * 128: c0 + (tt + 1) * 128,
                                dh * 512:(dh + 1) * 512,
                            ],
                            ot[:],
                        )
                c0 += NC

    nc.compile()
    return nc


def route_host(xf: np.ndarray, gate_w: np.ndarray):
    """Top-2 routing, bit-exact with the reference (jax on CPU)."""
    import jax
    import jax.numpy as jnp

    cpu = jax.devices("cpu")[0]
    with jax.default_device(cpu):
        xj = jax.device_put(xf, cpu)
        gj = jax.device_put(gate_w, cpu)
        probs = jax.nn.softmax(xj @ gj, axis=-1)
        vals, idx = jax.lax.top_k(probs, TOP_K)
        w = vals / jnp.sum(vals, axis=-1, keepdims=True)
    return np.asarray(idx), np.asarray(w)


def prepare_dispatch(x, gate_w):
    """Host routing + per-expert gather lists."""
    xf = np.ascontiguousarray(np.asarray(x).reshape(T, D), dtype=np.float32)
    gate_w = np.asarray(gate_w, dtype=np.float32)
    idx, w = route_host(xf, gate_w)
    tok_flat = np.repeat(np.arange(T), TOP_K)
    idx_flat = idx.ravel()
    w_flat = w.astype(np.float32).ravel()
    toks = []
    wts_list = []
    for e in range(E):
        sel = idx_flat == e
        toks.append(tok_flat[sel])
        wts_list.append(w_flat[sel])
    max_n = max(len(t) for t in toks)
    C = max(256, ((max_n + 127) // 128) * 128)
    return xf, toks, wts_list, C


def make_in_maps(xf, toks, wts_list, C, w1, w2, w3):
    xf_bf = xf.astype(BF16)
    in_maps = []
    for e in range(E):
        n_e = len(toks[e])
        xgT = np.zeros((D, C), dtype=BF16)
        xgT[:, :n_e] = xf_bf[toks[e]].T
        wflat = np.zeros(C, dtype=np.float32)
        wflat[:n_e] = wts_list[e]
        wts = np.ascontiguousarray(wflat.reshape(C // 128, 128).T)
        in_maps.append(
            {
                "xgT": xgT,
                "w1": np.asarray(w1[e], dtype=np.float32).astype(BF16),
                "w3": np.asarray(w3[e], dtype=np.float32).astype(BF16),
                "w2": np.asarray(w2[e], dtype=np.float32).astype(BF16),
                "wts": wts,
            }
        )
    return in_maps


def combine_outputs(results, toks):
    out = np.zeros((T, D), dtype=np.float32)
    for e in range(E):
        n_e = len(toks[e])
        out[toks[e]] += np.asarray(results[e]["yg"][:n_e], dtype=np.float32)
    return out.reshape(B, S, D)


def run(x, gate_w, w1, w2, w3, **spmd_kwargs):
    """Run the MoE. Returns (output, BassKernelResults)."""
    from concourse import bass_utils

    xf, toks, wts_list, C = prepare_dispatch(x, gate_w)
    if C not in _nc_cache:
        _nc_cache[C] = build_expert_ffn(C)
    nc = _nc_cache[C]

    in_maps = make_in_maps(xf, toks, wts_list, C, w1, w2, w3)
    res = bass_utils.run_bass_kernel_spmd(
        nc, in_maps, core_ids=list(range(E)), **spmd_kwargs
    )
    out = combine_outputs(res.results, toks).astype(np.asarray(x).dtype, copy=False)
    return out, res


def kernel(x, gate_w, w1, w2, w3):
    out, _ = run(x, gate_w, w1, w2, w3)
    return out


# revision 20
# speedup vs baseline: 1.2109x; 1.0063x over previous
"""MoE (top-2 of 8 experts, SwiGLU FFN) on 8 Trainium2 NeuronCores.

Strategy: expert-parallel. The gate/top-k routing is computed on host
(bit-exact with the reference: jax on CPU), tokens are dispatched to the
core owning their expert (sharding by top-k index), each core runs a
dense SwiGLU FFN over its gathered tokens (bf16 matmuls, fp32
accumulation) and scales rows by the renormalized top-k weight. The host
scatter-adds the per-expert partial outputs into the full [B,S,D] output.

Problem dims (hardcoded): B=4, S=2048, D=1024, E=8, TOP_K=2, H=3072.

SBUF budget per partition (bytes): w1+w3 96K, w2 48K, g 24K, xg 8K,
sil 4K, ot 4K  -> ~184K of 192K.
PSUM: ps1 x2 + ps3 x2 (stage A) + pso x4 (stage B) = 8 banks.
"""

import sys

if "/opt/trn_rl_repo" not in sys.path:
    sys.path.insert(0, "/opt/trn_rl_repo")

import numpy as np
import ml_dtypes

B, S, D = 4, 2048, 1024
E = 8
TOP_K = 2
H = 3 * D
T = B * S

BF16 = ml_dtypes.bfloat16

_nc_cache: dict = {}


def build_expert_ffn(C: int):
    """Bass program for one core: dense SwiGLU FFN over C gathered tokens.

    Inputs (per core e):
      xgT [D, C]  bf16 : gathered tokens, transposed, zero-padded
      w1  [D, H]  bf16
      w3  [D, H]  bf16
      w2  [H, D]  bf16
      wts [C, 1]  f32  : per-token combine weight (0 for padding)
    Output:
      yg  [C, D]  f32  : wts * (silu(xg@w1) * (xg@w3)) @ w2
    """
    import concourse.bacc as bacc
    import concourse.tile as tile
    import concourse.mybir as mybir

    fp32 = mybir.dt.float32
    bf16 = mybir.dt.bfloat16

    assert C % 128 == 0
    # token chunks (PSUM bank free dim <= 512); keep >=256 where possible so
    # the [128,128] LDWEIGHTS stays hidden under the matmul stream
    chunks = []
    rem = C
    while rem > 640:
        chunks.append(512)
        rem -= 512
    if rem <= 512:
        chunks.append(rem)
    else:
        chunks.extend([rem - 256, 256])
    KD = D // 128                # 8  k-tiles over D
    NH = H // 128                # 24 h-tiles over H
    ND = D // 512                # 2  512-wide output column tiles

    nc = bacc.Bacc("TRN2", target_bir_lowering=False, debug=False, num_devices=8)

    # tokens pre-packed on host: xgk[p, k*C + c] = x_bf16[token c, k*128 + p]
    xgk = nc.dram_tensor("xgk", [128, KD * C], bf16, kind="ExternalInput")
    # w1 and w3 fused along the free dim: w13[d, :H] = w1[d], w13[d, H:] = w3[d]
    w13 = nc.dram_tensor("w13", [D, 2 * H], bf16, kind="ExternalInput")
    w2 = nc.dram_tensor("w2", [H, D], bf16, kind="ExternalInput")
    # combine weights pre-tiled on host: wts[p, n] = weight of token n*128+p
    wts = nc.dram_tensor("wts", [128, C // 128], fp32, kind="ExternalInput")
    yg = nc.dram_tensor("yg", [C, D], fp32, kind="ExternalOutput")

    with tile.TileContext(nc) as tc:
        with (
            tc.tile_pool(name="wres", bufs=1) as wres,
            tc.tile_pool(name="xgp", bufs=1) as xgp,
            tc.tile_pool(name="gp", bufs=1) as gp,
            tc.tile_pool(name="tmp", bufs=2) as tmp,
            tc.tile_pool(name="outp", bufs=2) as outp,
            tc.tile_pool(name="psA", bufs=2, space="PSUM") as psA,
            tc.tile_pool(name="psB", bufs=4, space="PSUM") as psB,
        ):
            # DMA issue order matters: the first chunk's tokens and the
            # combine weights go first (gpsimd/SWDGE queue), then w1/w3
            # (needed by stage A from t~0), then w2 (needed ~80us in).
            xgk_3d = xgk.ap().rearrange("p (k c) -> p k c", k=KD)

            def load_xg_chunk(c0, NC, eng=None):
                eng = eng or nc.gpsimd
                # one DMA per chunk: [128, KD, NC] (k stride C, c contiguous)
                xt = xgp.tile([128, KD * NC], bf16, tag="xg")
                eng.dma_start(
                    xt[:].rearrange("p (k c) -> p k c", k=KD),
                    xgk_3d[:, :, c0:c0 + NC],
                )
                return xt

            # DMA ordering: all queues stripe over the same 16 HW engines, so
            # what matters is issue order by first use. gpsimd: wts + chunk-0
            # tokens (small, early). sync: w1/w3 (stage A, critical from t=0),
            # then w2 (stage B, needed ~80us in).
            wts_sb = wres.tile([128, C // 128], fp32, tag="wts")
            nc.gpsimd.dma_start(wts_sb[:], wts.ap())
            xg0_sb = load_xg_chunk(0, chunks[0])

            w13_sb = []
            for k in range(KD):
                t1 = wres.tile([128, 2 * H], bf16, tag=f"w13_{k}")
                nc.sync.dma_start(t1[:], w13.ap()[k * 128:(k + 1) * 128, :])
                w13_sb.append(t1)
            w2_sb = []
            for ht in range(NH):
                t2 = wres.tile([128, D], bf16, tag=f"w2_{ht}")
                nc.sync.dma_start(t2[:], w2.ap()[ht * 128:(ht + 1) * 128, :])
                w2_sb.append(t2)

            c0 = 0
            for ch, NC in enumerate(chunks):
                NT = NC // 128
                xg_t = xg0_sb if ch == 0 else load_xg_chunk(c0, NC)
                xg_sb = [xg_t[:, k * NC:(k + 1) * NC] for k in range(KD)]

                # stage A: gT[h, tok] = silu(y1) * y3 for all 24 h-tiles
                g_tiles = []
                for ht in range(NH):
                    # chunk 0: stage B is idle, so borrow psB's banks for
                    # extra in-flight groups while w1/w3 are still arriving
                    pool = psB if (ch == 0 and ht % 2 == 1) else psA
                    ps1 = pool.tile([128, NC], fp32, tag="ps1" if pool is psA else "pso")
                    ps3 = pool.tile([128, NC], fp32, tag="ps3" if pool is psA else "pso")
                    for k in range(KD):
                        nc.tensor.matmul(
                            ps1[:],
                            w13_sb[k][:, ht * 128:(ht + 1) * 128],
                            xg_sb[k],
                            start=(k == 0),
                            stop=(k == KD - 1),
                        )
                    for k in range(KD):
                        nc.tensor.matmul(
                            ps3[:],
                            w13_sb[k][:, H + ht * 128:H + (ht + 1) * 128],
                            xg_sb[k],
                            start=(k == 0),
                            stop=(k == KD - 1),
                        )
                    sig = tmp.tile([128, NC], fp32, tag="sig")
                    nc.scalar.activation(
                        sig[:], ps1[:], mybir.ActivationFunctionType.Sigmoid
                    )
                    sil = tmp.tile([128, NC], fp32, tag="sil")
                    nc.vector.tensor_mul(sil[:], sig[:], ps1[:])
                    gt = gp.tile([128, NC], bf16, tag=f"g_{ht}")
                    nc.vector.tensor_mul(gt[:], sil[:], ps3[:])
                    g_tiles.append(gt)

                # stage B: yg[tok, d] = wts[tok] * (g.T @ w2)
                for tt in range(NT):
                    gtile_idx = c0 // 128 + tt
                    for dh in range(ND):
                        pso = psB.tile([128, 512], fp32, tag="pso")
                        for ht in range(NH):
                            nc.tensor.matmul(
                                pso[:],
                                g_tiles[ht][:, tt * 128:(tt + 1) * 128],
                                w2_sb[ht][:, dh * 512:(dh + 1) * 512],
                                start=(ht == 0),
                                stop=(ht == NH - 1),
                            )
                        ot = outp.tile([128, 512], fp32, tag="ot")
                        nc.vector.tensor_scalar_mul(
                            ot[:], pso[:], wts_sb[:, gtile_idx:gtile_idx + 1]
                        )
                        nc.sync.dma_start(
                            yg.ap()[
                                c0 + tt * 128: c0 + (tt + 1) * 128,
                                dh * 512:(dh + 1) * 512,
                            ],
                            ot[:],
                        )
                c0 += NC

    nc.compile()
    return nc


def route_host(xf: np.ndarray, gate_w: np.ndarray):
    """Top-2 routing, bit-exact with the reference (jax on CPU)."""
    import jax
    import jax.numpy as jnp

    cpu = jax.devices("cpu")[0]
    with jax.default_device(cpu):
        xj = jax.device_put(xf, cpu)
        gj = jax.device_put(gate_w, cpu)
        probs = jax.nn.softmax(xj @ gj, axis=-1)
        vals, idx = jax.lax.top_k(probs, TOP_K)
        w = vals / jnp.sum(vals, axis=-1, keepdims=True)
    return np.asarray(idx), np.asarray(w)


def prepare_dispatch(x, gate_w):
    """Host routing + per-expert gather lists."""
    xf = np.ascontiguousarray(np.asarray(x).reshape(T, D), dtype=np.float32)
    gate_w = np.asarray(gate_w, dtype=np.float32)
    idx, w = route_host(xf, gate_w)
    tok_flat = np.repeat(np.arange(T), TOP_K)
    idx_flat = idx.ravel()
    w_flat = w.astype(np.float32).ravel()
    toks = []
    wts_list = []
    for e in range(E):
        sel = idx_flat == e
        toks.append(tok_flat[sel])
        wts_list.append(w_flat[sel])
    max_n = max(len(t) for t in toks)
    C = max(256, ((max_n + 127) // 128) * 128)
    return xf, toks, wts_list, C


def make_in_maps(xf, toks, wts_list, C, w1, w2, w3):
    xf_bf = xf.astype(BF16)
    in_maps = []
    for e in range(E):
        n_e = len(toks[e])
        xgT = np.zeros((D, C), dtype=BF16)
        xgT[:, :n_e] = xf_bf[toks[e]].T
        # pack [D, C] -> [128, KD*C]: row p holds k-tiles back to back
        xgk = np.ascontiguousarray(
            xgT.reshape(D // 128, 128, C).transpose(1, 0, 2).reshape(128, -1)
        )
        wflat = np.zeros(C, dtype=np.float32)
        wflat[:n_e] = wts_list[e]
        wts = np.ascontiguousarray(wflat.reshape(C // 128, 128).T)
        w13 = np.concatenate(
            [
                np.asarray(w1[e], dtype=np.float32).astype(BF16),
                np.asarray(w3[e], dtype=np.float32).astype(BF16),
            ],
            axis=1,
        )
        in_maps.append(
            {
                "xgk": xgk,
                "w13": w13,
                "w2": np.asarray(w2[e], dtype=np.float32).astype(BF16),
                "wts": wts,
            }
        )
    return in_maps


def combine_outputs(results, toks):
    out = np.zeros((T, D), dtype=np.float32)
    for e in range(E):
        n_e = len(toks[e])
        out[toks[e]] += np.asarray(results[e]["yg"][:n_e], dtype=np.float32)
    return out.reshape(B, S, D)


def run(x, gate_w, w1, w2, w3, **spmd_kwargs):
    """Run the MoE. Returns (output, BassKernelResults)."""
    from concourse import bass_utils

    xf, toks, wts_list, C = prepare_dispatch(x, gate_w)
    if C not in _nc_cache:
        _nc_cache[C] = build_expert_ffn(C)
    nc = _nc_cache[C]

    in_maps = make_in_maps(xf, toks, wts_list, C, w1, w2, w3)
    res = bass_utils.run_bass_kernel_spmd(
        nc, in_maps, core_ids=list(range(E)), **spmd_kwargs
    )
    out = combine_outputs(res.results, toks).astype(np.asarray(x).dtype, copy=False)
    return out, res


def kernel(x, gate_w, w1, w2, w3):
    out, _ = run(x, gate_w, w1, w2, w3)
    return out
